# revision 74
# baseline (speedup 1.0000x reference)
"""Trainium2 Bass kernel for causal multi-head attention (B=2, T=2048, D=1024, H=16).

Sharding: 8 cores = 2 batches x 4 head-groups. Each core computes 4 heads
(as 2 head-pairs packed into 128 partitions) of one batch, plus its row-shard
of the output projection; the host sums the 4 partial outputs per batch.

Compute is bf16 with fp32 PSUM accumulation. Softmax uses no max-subtraction
(scores ~ N(0,1), exp is safe in fp32) and gets the denominator for free via
an all-ones column block prepended to V.

Pipeline design (PE p-state aware): the TRN2 PE only reaches full clock after
3us of continuous execution, so the tensor queue must never stall. The causal
mask is additive (-87), folded into the scores PSUM by a small matmul
(mask^T stationary, identity streaming) so no other engine sits between the
exp and the AV^T matmul. AV^T lags the scores by 2 k-blocks so the exp
latency (ScalarE) is fully hidden. ScalarE runs only exp; Q/K bias-adds and
all PSUM->SBUF casts live on VectorE. Output-projection chunks for q-tile i
are sprinkled into q-tile i+1's k-loop as PE filler.
"""

import numpy as np
import ml_dtypes
from contextlib import ExitStack

import concourse.bass as bass
import concourse.mybir as mybir
import concourse.tile as tile
from concourse import bacc
from concourse.bass_utils import run_bass_kernel_spmd

BF16 = mybir.dt.bfloat16
F32 = mybir.dt.float32
AF = mybir.ActivationFunctionType
bf16 = ml_dtypes.bfloat16

B, T, D, H, DH = 2, 2048, 1024, 16, 64
NCORES = 8
QTILE = 512          # q columns per score tile
NQT = T // QTILE     # 4
TCH = T // 128       # 16 t-chunks / k-blocks
MASKVAL = -87.0      # additive causal mask; exp(-87 + s) ~ 0 in fp32

_CACHE = {}


def _build():
    nc = bacc.Bacc(
        "TRN2", target_bir_lowering=False, debug=False, num_devices=NCORES
    )
    # wb columns: [wq0|wq1|wk0|wk1|wv0|wv1|maskT|ident] (8x128), one DMA
    xt_d = nc.dram_tensor("xt", [256, T], BF16, kind="ExternalInput").ap()
    wb_d = nc.dram_tensor("wb", [128, 1024], BF16, kind="ExternalInput").ap()
    wo_d = nc.dram_tensor("wo", [128, 2 * D], BF16, kind="ExternalInput").ap()
    bias_d = nc.dram_tensor("bias", [128, 4], F32, kind="ExternalInput").ap()
    y_d = nc.dram_tensor("y", [T, D], BF16, kind="ExternalOutput").ap()

    with tile.TileContext(nc) as tc, ExitStack() as ctx:
        const = ctx.enter_context(tc.tile_pool(name="const", bufs=1))
        pers = ctx.enter_context(tc.tile_pool(name="pers", bufs=1))
        pex = ctx.enter_context(tc.tile_pool(name="pex", bufs=2))
        pdiv = ctx.enter_context(tc.tile_pool(name="pdiv", bufs=4))
        pysb = ctx.enter_context(tc.tile_pool(name="pysb", bufs=4))
        ps_s = ctx.enter_context(tc.tile_pool(name="ps_s", bufs=2, space="PSUM"))
        ps_a = ctx.enter_context(tc.tile_pool(name="ps_a", bufs=2, space="PSUM"))
        ps_m = ctx.enter_context(tc.tile_pool(name="ps_m", bufs=2, space="PSUM"))

        # Consolidated input loads. gpsimd issues DMAs nearly for free
        # (hardware DGE); sync carries the other xt half + output stores.
        wb_sb = const.tile([128, 1024], BF16, tag="wb", name="wb_sb")
        # Parallelize the two startup-critical loads across DGE paths: the
        # first-group weights (wq0, wq1, wk0) ride sync's hardware DGE while
        # xt0 j3 rides gpsimd's software DGE — both land ~2us earlier than
        # queued behind each other.
        nc.sync.dma_start(wb_sb[:, 0:384], wb_d[:, 0:384])
        bias_sb = const.tile([128, 4], F32, tag="bias", name="bias_sb")
        wq_sb = [wb_sb[:, 128 * p : 128 * (p + 1)] for p in range(2)]
        wk_sb = [wb_sb[:, 256 + 128 * p : 256 + 128 * (p + 1)] for p in range(2)]
        wv_sb = [wb_sb[:, 512 + 128 * p : 512 + 128 * (p + 1)] for p in range(2)]
        msk_sb = wb_sb[:, 768:896]
        idn_sb = wb_sb[:, 896:1024]
        bq_sb = [bias_sb[:, p : p + 1] for p in range(2)]
        bk_sb = [bias_sb[:, 2 + p : 3 + p] for p in range(2)]

        # gpsimd's hardware DGE issues DMAs with the least latency: give it the
        # startup-critical chunks (xt0 j3 for the first Q tile, j0 for K/V).
        xt_sb = [
            pers.tile([128, T], BF16, tag=f"xt{p}", name=f"xt{p}_sb")
            for p in range(2)
        ]

        def xt_dma(eng, p, j):
            sl = slice(QTILE * j, QTILE * (j + 1))
            eng.dma_start(xt_sb[p][:, sl], xt_d[128 * p : 128 * (p + 1), sl])

        xt_dma(nc.gpsimd, 0, 3)
        nc.gpsimd.dma_start(bias_sb[:], bias_d)
        xt_dma(nc.sync, 0, 0)
        nc.gpsimd.dma_start(wb_sb[:, 384:1024], wb_d[:, 384:1024])
        for j in (1, 2):
            xt_dma(nc.sync, 0, j)
        for j in (3, 0, 1, 2):
            xt_dma(nc.gpsimd, 1, j)

        wof_sb = const.tile([128, 2 * D], BF16, tag="wo", name="wof_sb")
        nc.gpsimd.dma_start(wof_sb[:], wo_d)
        wo_sb = [wof_sb[:, D * p : D * (p + 1)] for p in range(2)]

        qT, kT, vaug, attnT = [], [], [], []
        for p in range(2):
            qT.append(pers.tile([128, T], BF16, tag=f"qT{p}", name=f"qT{p}_sb"))
            kT.append(pers.tile([128, T], BF16, tag=f"kT{p}", name=f"kT{p}_sb"))
            vaug.append(
                pers.tile([128, 256 * TCH], BF16, tag=f"va{p}", name=f"va{p}_sb")
            )
            attnT.append(
                pers.tile([128, T], BF16, tag=f"aT{p}", name=f"aT{p}_sb")
            )

        # ---- Phase A: QKV projections (2-head block-diagonal packing) ----
        def qk_piece(p, j, which, add_eng="v"):
            sl = slice(QTILE * j, QTILE * (j + 1))
            if which == "q":
                pq = ps_m.tile([128, QTILE], F32, tag="m", name="pq")
                nc.tensor.matmul(
                    pq[:], wq_sb[p][:], xt_sb[p][:, sl], start=True, stop=True
                )
                nc.vector.tensor_scalar_add(qT[p][:, sl], pq[:], bq_sb[p][:])
            elif add_eng == "s":
                # Identity shares every act-table set with Exp: no table swap
                pk = ps_m.tile([128, QTILE], F32, tag="m", name="pk")
                nc.tensor.matmul(
                    pk[:], wk_sb[p][:], xt_sb[p][:, sl], start=True, stop=True
                )
                nc.scalar.activation(
                    kT[p][:, sl], pk[:], AF.Identity, bias=bk_sb[p][:]
                )
            else:
                pk = ps_m.tile([128, QTILE], F32, tag="m", name="pk")
                nc.tensor.matmul(
                    pk[:], wk_sb[p][:], xt_sb[p][:, sl], start=True, stop=True
                )
                nc.vector.tensor_scalar_add(kT[p][:, sl], pk[:], bk_sb[p][:])

        def v_piece(p, c4):  # one psum tile = 4 t-chunks of V
            pv = ps_m.tile([128, QTILE], F32, tag="m", name="pv")
            for j in range(4):
                tj = 4 * c4 + j
                nc.tensor.matmul(
                    pv[:, 128 * j : 128 * (j + 1)],
                    xt_sb[p][:, 128 * tj : 128 * (tj + 1)],
                    wv_sb[p][:],
                    start=True,
                    stop=True,
                )
            src = pv.rearrange("p (c h x) -> p c h x", c=4, h=2)
            dst = vaug[p][:, 1024 * c4 : 1024 * (c4 + 1)].rearrange(
                "p (c h x) -> p c h x", c=4, h=2
            )[:, :, :, 64:128]
            nc.vector.tensor_copy(dst, src)

        def phase_a_prefix(p):
            # Just enough QKV for q-tile 3's k-loop to start: qT j3, kT j0,
            # V t-chunks 0-3. The rest rides in the k-loop's PE slack.
            # V_aug layout per (t-chunk, head): [ones x 64 | V_h] (M=128 each).
            # The ones half makes the AV^T matmul emit the softmax denominator
            # replicated across partitions 0:64 of its output (where the
            # custom-DVE reciprocal can read it directly).
            nc.gpsimd.memset(
                vaug[p].rearrange("p (g c) -> p g c", c=128)[:, :, 0:64], 1.0
            )
            qk_piece(p, 3, "q")
            qk_piece(p, 0, "k", add_eng="s")

        def phase_a_hooks(p):
            # deadlines (k-loop iter in the qi=3 group): K j needed at iter
            # 4j, V chunk c4 at iter 4*c4+2; Q j0-j2 only by later groups.
            return {
                0: lambda: v_piece(p, 0),
                1: lambda: qk_piece(p, 1, "k"),
                2: lambda: v_piece(p, 1),
                4: lambda: qk_piece(p, 2, "k"),
                6: lambda: v_piece(p, 2),
                8: lambda: qk_piece(p, 3, "k"),
                10: lambda: v_piece(p, 3),
                11: lambda: qk_piece(p, 0, "q"),
                12: lambda: qk_piece(p, 1, "q"),
                13: lambda: qk_piece(p, 2, "q"),
            }

        # ---- Phase B: attention + output projection ----
        def avt_pair(p, qi, kb, nkb, aps, es_t, hs=(0, 1)):
            # out^T = [1 | V]^T @ es accumulated over k-blocks: rows 0:64 are
            # the softmax denominator replicated 64x, rows 64:128 attn@V
            # transposed.
            cs = max(0, 128 * (kb - 4 * qi))
            for h in hs:
                nc.tensor.matmul(
                    aps[h][:, cs:QTILE],
                    vaug[p][:, 256 * kb + 128 * h : 256 * kb + 128 * (h + 1)],
                    es_t[:, 1024 * kb + 512 * h + cs : 1024 * kb + 512 * (h + 1)],
                    start=(kb == 0),
                    stop=(kb == nkb - 1),
                )

        def outproj_mm(tj, half, pool=None):
            # matmul half of the output projection for a 128-row t-chunk
            tsl = slice(128 * tj, 128 * (tj + 1))
            nsl = slice(512 * half, 512 * (half + 1))
            yp = (pool or ps_m).tile([128, 512], F32, tag="m" if pool is None
                                     else "a", name=f"y{half}")
            nc.tensor.matmul(
                yp[:], attnT[0][:, tsl], wo_sb[0][:, nsl],
                start=True, stop=False,
            )
            nc.tensor.matmul(
                yp[:], attnT[1][:, tsl], wo_sb[1][:, nsl],
                start=False, stop=True,
            )
            return yp

        def outproj_cast(yp, half, ysb, cast_eng="v"):
            nsl = slice(512 * half, 512 * (half + 1))
            if cast_eng == "s":
                nc.scalar.activation(ysb[:, nsl], yp[:], AF.Copy)
            else:
                nc.vector.tensor_copy(ysb[:, nsl], yp[:])

        def outproj_half(tj, half, ysb, cast_eng="v", pool=None):
            yp = outproj_mm(tj, half, pool)
            outproj_cast(yp, half, ysb, cast_eng)

        def y_dma(tj, ysb, cols=slice(0, D), eng=None):
            tsl = slice(128 * tj, 128 * (tj + 1))
            if eng is None:
                eng = nc.sync if tj % 2 == 0 else nc.gpsimd
            eng.dma_start(y_d[tsl, cols], ysb[:, cols])

        def outproj_tj(tj, final=False, cast_eng="v"):
            # In the final burst the idle ScalarE takes half the casts, the
            # half-row DMAs go out as soon as their cast lands, and the dead
            # aps banks double the PSUM pipeline depth.
            ysb = pysb.tile([128, D], BF16, tag="y", name="ysb")
            if final:
                # last two chunks' second halves ride sync: its hardware DGE
                # pipelines the issue, while gpsimd's ~1us software descriptor
                # generation would delay the transfers gating the exit barrier
                pool = ps_a if tj % 2 == 1 else None
                h1_eng = nc.sync if tj >= 2 else nc.gpsimd
                outproj_half(tj, 0, ysb, cast_eng="s", pool=pool)
                y_dma(tj, ysb, slice(0, 512), nc.sync)
                outproj_half(tj, 1, ysb, pool=pool)
                y_dma(tj, ysb, slice(512, 1024), h1_eng)
            else:
                outproj_half(tj, 0, ysb, cast_eng=cast_eng)
                outproj_half(tj, 1, ysb, cast_eng=cast_eng)
                y_dma(tj, ysb)

        pending_tail = [None]

        def run_group(p, qi, mid_hooks, tail_hook, tail_cast="v",
                      tail_filler=None):
            # mid_hooks: {kb: callable} PE filler emitted inside the k-loop;
            # tail_hook: callable emitted between the two tail AV^T pairs
            # (covers the last exp's latency). The previous group's tail
            # (pending AV^T pairs + divisions) is emitted AFTER this group's
            # first two k-iterations, which touch no aps/attnT state, so the
            # scalar exp stream never starves across the group boundary.
            q0 = QTILE * qi
            nkb = 4 * (qi + 1)
            es_t = pex.tile([128, 1024 * nkb], BF16, tag="es", name="es_t")
            aps = [
                ps_a.tile([128, QTILE], F32, tag="a", name=f"aps{h}")
                for h in range(2)
            ]

            def iter_kb(kb):
                cs = max(0, 128 * (kb - 4 * qi))
                k0 = 128 * kb
                sps = ps_s.tile([128, 1024], F32, tag="s", name="sps")
                for h in range(2):
                    hp = slice(64 * h, 64 * (h + 1))
                    nc.tensor.matmul(
                        sps[:, 512 * h + cs : 512 * (h + 1)],
                        kT[p][hp, k0 : k0 + 128],
                        qT[p][hp, q0 + cs : q0 + QTILE],
                        start=True,
                        stop=True,
                    )
                if kb >= 4 * qi:  # diagonal block: add -87 above the diagonal
                    for h in range(2):
                        nc.tensor.matmul(
                            sps[:, 512 * h + cs : 512 * h + cs + 128],
                            msk_sb,
                            idn_sb,
                            start=False,
                            stop=True,
                        )
                if cs == 0:  # contiguous full block: keep the AP flat
                    nc.scalar.activation(
                        es_t[:, 1024 * kb : 1024 * (kb + 1)], sps[:], AF.Exp
                    )
                else:
                    nc.scalar.activation(
                        es_t[:, 1024 * kb : 1024 * (kb + 1)].rearrange(
                            "p (h x) -> p h x", h=2
                        )[:, :, cs:512],
                        sps.rearrange("p (h x) -> p h x", h=2)[:, :, cs:512],
                        AF.Exp,
                    )
                if kb >= 2:
                    avt_pair(p, qi, kb - 2, nkb, aps, es_t)
                if kb in mid_hooks:
                    mid_hooks[kb]()

            ysb_box = []

            def tail():
                # Last AV^T pair interleaved with the division chain so the
                # aps banks release as early as possible (the next group's
                # first AV^T reuses them). The boundary outproj's matmuls are
                # emitted BEFORE the division muls: they read other attnT
                # columns, and emitting them first avoids a false
                # emission-order dependency that would idle the PE behind the
                # VectorE division chain.
                avt_pair(p, qi, nkb - 2, nkb, aps, es_t)
                avt_pair(p, qi, nkb - 1, nkb, aps, es_t, hs=(0,))
                rec0 = pdiv.tile([64, QTILE], F32, tag="rec", name="rec")
                nc.vector.reciprocal_approx_fast(rec0[:], aps[0][0:64, :])
                avt_pair(p, qi, nkb - 1, nkb, aps, es_t, hs=(1,))
                rec1 = pdiv.tile([64, QTILE], F32, tag="rec", name="rec")
                nc.vector.reciprocal_approx_fast(rec1[:], aps[1][0:64, :])
                yp = None
                if tail_hook is not None:
                    ysb = pysb.tile([128, D], BF16, tag="y", name="ysb")
                    ysb_box.append(ysb)
                    yp = outproj_mm(tail_hook, 0)
                if tail_filler is not None:
                    tail_filler()
                nc.vector.tensor_mul(
                    attnT[p][0:64, q0 : q0 + QTILE], aps[0][64:128, :], rec0[:]
                )
                nc.vector.tensor_mul(
                    attnT[p][64:128, q0 : q0 + QTILE], aps[1][64:128, :], rec1[:]
                )
                if tail_hook is not None:
                    outproj_cast(yp, 0, ysb_box[0], cast_eng=tail_cast)

            def post():
                if tail_hook is not None:
                    outproj_half(tail_hook, 1, ysb_box[0], cast_eng=tail_cast)
                    y_dma(tail_hook, ysb_box[0])

            iter_kb(0)
            iter_kb(1)
            if pending_tail[0] is not None:
                tail_fn, post_fn = pending_tail[0]
                tail_fn()
            else:
                post_fn = None
            for kb in range(2, nkb):
                iter_kb(kb)
                if kb == 2 and post_fn is not None:
                    post_fn()
            pending_tail[0] = (tail, post)

        # q-tiles in descending size order: the big (scalar-exp-bound) groups
        # run first; the last group leaves only a 4-block exp drain before the
        # final output-projection burst.
        phase_a_prefix(0)
        run_group(0, 3, phase_a_hooks(0), None)
        phase_a_prefix(1)
        run_group(1, 3, phase_a_hooks(1), None)
        for qi in (2, 1, 0):
            t0 = 4 * (qi + 1)  # previous (larger) q-tile's chunks
            if qi > 0:
                # hooks at kb 4/6: clear of the boundary flush + post block at
                # kb 2-3, so the PE filler doesn't outrun the exp buffer
                run_group(
                    0, qi,
                    {4: (lambda t=t0: outproj_tj(t)),
                     6: (lambda t=t0 + 1: outproj_tj(t))},
                    t0 + 2,
                )
                run_group(1, qi, {}, t0 + 3)
            else:
                run_group(0, 0,
                          {3: (lambda t=t0: outproj_tj(t, cast_eng="s"))},
                          t0 + 2, tail_cast="s")
                # prestart the final burst's first chunk's attnT[0]-half
                # matmuls inside the last group's tail: they only need the
                # p0 divisions (done one group earlier) and fill the PE
                # while the last division chain drains on VectorE
                prestart = []

                def prestart_tj0():
                    for half in range(2):
                        nsl = slice(512 * half, 512 * (half + 1))
                        yp = ps_m.tile([128, 512], F32, tag="m",
                                       name=f"y{half}")
                        nc.tensor.matmul(
                            yp[:], attnT[0][:, 0:128], wo_sb[0][:, nsl],
                            start=True, stop=False,
                        )
                        prestart.append(yp)

                run_group(1, 0,
                          {3: (lambda t=t0 + 1: outproj_tj(t, cast_eng="s"))},
                          t0 + 3, tail_cast="s", tail_filler=prestart_tj0)
        tail_fn, post_fn = pending_tail[0]
        tail_fn()
        post_fn()
        # final burst; chunk 0 completes the prestarted accumulations
        ysb0 = pysb.tile([128, D], BF16, tag="y", name="ysb")
        for half in range(2):
            nsl = slice(512 * half, 512 * (half + 1))
            nc.tensor.matmul(
                prestart[half][:], attnT[1][:, 0:128], wo_sb[1][:, nsl],
                start=False, stop=True,
            )
        nc.scalar.activation(ysb0[:, 0:512], prestart[0][:], AF.Copy)
        y_dma(0, ysb0, slice(0, 512), nc.sync)
        nc.vector.tensor_copy(ysb0[:, 512:1024], prestart[1][:])
        y_dma(0, ysb0, slice(512, 1024), nc.gpsimd)
        for tj in range(1, 4):
            outproj_tj(tj, final=True)

    nc.compile()
    return nc


def _host_prep(x, Wq, bq, Wk, bk, Wv, bv, Wo, bo):
    x = np.asarray(x, np.float32)
    Wq, bq = np.asarray(Wq, np.float32), np.asarray(bq, np.float32)
    Wk, bk = np.asarray(Wk, np.float32), np.asarray(bk, np.float32)
    Wv, bv = np.asarray(Wv, np.float32), np.asarray(bv, np.float32)
    Wo, bo = np.asarray(Wo, np.float32), np.asarray(bo, np.float32)
    # additive-mask matmul: psum[i,j] += sum_p lhsT[p,i]*I[p,j] = lhsT[j,i],
    # want MASKVAL where k>q i.e. i>j  =>  lhsT = MASKVAL*triu(ones, 1)
    mskT = MASKVAL * np.triu(np.ones((128, 128), np.float32), 1)
    ident = np.eye(128, dtype=np.float32)
    in_maps = []
    for c in range(NCORES):
        b, g = divmod(c, 4)
        h0 = 4 * g
        xt = np.ascontiguousarray(x[b, :, 256 * g : 256 * (g + 1)].T).astype(bf16)
        wb = np.zeros((128, 1024), np.float32)
        bias = np.zeros((128, 4), np.float32)
        for p in range(2):
            ha, hb = h0 + 2 * p, h0 + 2 * p + 1
            wb[0:64, 128 * p : 128 * p + 64] = Wq[ha] * 0.125
            wb[64:128, 128 * p + 64 : 128 * p + 128] = Wq[hb] * 0.125
            wb[0:64, 256 + 128 * p : 256 + 128 * p + 64] = Wk[ha]
            wb[64:128, 256 + 128 * p + 64 : 256 + 128 * p + 128] = Wk[hb]
            wb[0:64, 512 + 128 * p : 512 + 128 * p + 64] = Wv[ha]
            wb[64:128, 512 + 128 * p + 64 : 512 + 128 * p + 128] = Wv[hb]
            bias[0:64, p] = bq[ha] * 0.125
            bias[64:128, p] = bq[hb] * 0.125
            bias[0:64, 2 + p] = bk[ha]
            bias[64:128, 2 + p] = bk[hb]
        wb[:, 768:896] = mskT
        wb[:, 896:1024] = ident
        wo_c = np.ascontiguousarray(
            Wo[256 * g : 256 * (g + 1)].reshape(2, 128, D).transpose(1, 0, 2)
            .reshape(128, 2 * D)
        ).astype(bf16)
        in_maps.append(
            {"xt": xt, "wb": wb.astype(bf16), "wo": wo_c, "bias": bias}
        )
    # bv contributes bv_flat @ Wo to every output row (softmax weights sum to 1)
    bo_eff = bo + bv.reshape(-1) @ Wo
    return in_maps, bo_eff


def _finalize(results, bo_eff):
    out = np.zeros((B, T, D), np.float32)
    for c in range(NCORES):
        out[c // 4] += np.asarray(results[c]["y"], dtype=np.float32)
    out += bo_eff[None, None, :]
    return out


def kernel(**inputs):
    if "nc" not in _CACHE:
        _CACHE["nc"] = _build()
    nc = _CACHE["nc"]
    in_maps, bo_eff = _host_prep(**inputs)
    res = run_bass_kernel_spmd(
        nc, in_maps, core_ids=list(range(NCORES)), trace=False
    )
    return _finalize(res.results, bo_eff)


def kernel_traced(**inputs):
    """Dev helper: run with NTFF profiling, return (out, exec_time_ns, tmpdir)."""
    import glob
    import tempfile

    from concourse import bass2jax
    from trn_agent_boot.trn_boot import _ntff_profile_via_ctypes

    if "nc" not in _CACHE:
        _CACHE["nc"] = _build()
    nc = _CACHE["nc"]
    in_maps, bo_eff = _host_prep(**inputs)
    hook = _ntff_profile_via_ctypes("/opt/axon/libaxon_pjrt.so")
    tmpdir = tempfile.mkdtemp(prefix="mha_trace_")
    with hook(tmpdir, [0]):
        results = bass2jax.run_bass_via_pjrt(nc, in_maps, n_cores=NCORES)
    out = _finalize(results, bo_eff)

    exec_time_ns = None
    try:
        import gauge.profiler
        from concourse._compat import FishPath

        ntffs = glob.glob(f"{tmpdir}/*.ntff")
        if ntffs:
            profile = gauge.profiler.Profile(
                profile_path=FishPath(tmpdir),
                kernel_dev_mode=True,
                profile_on_exit=False,
                bass_kernel=nc.m,
                offline_processing=True,
                fname="*_body*",
            )
            pres = profile.to_perfetto(model_index=(0,))
            if pres:
                exec_time_ns = pres[0].exec_time_ns
    except Exception as e:  # profiling is best-effort
        print(f"profile processing failed: {type(e).__name__}: {e}")
    return out, exec_time_ns, tmpdir
